# revision 27
# baseline (speedup 1.0000x reference)
"""DiT block kernel for Trainium2, 8 NeuronCores, data-parallel over batch.

Mixed fp8(e4m3)/bf16 GEMMs chosen by measured error contribution:
  - q/k projections + scores run fully in fp8 with DoubleRow (2 contraction
    chunks of 128 per instruction = 2x PE throughput); softmax washes out
    the quantization noise, so this is accuracy-free.
  - exp output and v are stored fp8 (pointwise quantization averages out
    under the positive softmax weights) which lets PV run DoubleRow.
  - fc2 runs fp8+DoubleRow (h and W2 fp8) - W2 then fits SBUF persistently.
  - c6/adaLN, the v/out-proj/fc1 GEMMs stay bf16: their weight-quantization
    error passes straight to the output.
LayerNorm stats, softmax denominators, residual stream and output are fp32.

Layout per core (S=1024 tokens, H=1024 features, 16 heads x 64):
  - x_res [128, 8*1024] f32 token-major; xmT fp8 / xm2T,yT bf16 / qT,kT fp8
    all [128, 8, 1024] feature-chunk-major (transposed).
  - v_sb [128, 8, 1040] fp8 token-major; per head 65 cols [v0..v63, ones]
    with ones=1/64: a small DoubleRow matmul per head emits den/64 at psum
    partition 0, so the reciprocal chain runs at partition 0 (no DMA).
  - Head pair shares one [128,512] PV psum via tile_position rows 0/64.
  - MLP runs in token halves, fc1 of half 0 interleaved with the second
    attention query-half so PE work overlaps the ACT-bound exp phase.
"""

import os
import sys


def _ensure_path():
    for p in ("/opt/trn_rl_repo", "/root/.axon_site/_ro/trn_rl_repo"):
        if os.path.isdir(p) and p not in sys.path:
            sys.path.insert(0, p)


_ensure_path()

import ml_dtypes  # noqa: E402
import numpy as np  # noqa: E402
from contextlib import ExitStack  # noqa: E402

import concourse.bass as bass  # noqa: E402
import concourse.tile as tile  # noqa: E402
from concourse import bacc, mybir  # noqa: E402
from concourse import bass_utils  # noqa: E402
from concourse.masks import make_identity  # noqa: E402

F32 = mybir.dt.float32
BF16 = mybir.dt.bfloat16
FP8 = mybir.dt.float8e4
AF = mybir.ActivationFunctionType
ALU = mybir.AluOpType
DR = mybir.MatmulPerfMode.DoubleRow

H = 1024
S = 1024
NH = 16
DH = 64
MLP = 4096
B = 8
EPS = 1e-5
P = 128
HC = H // P     # 8 feature chunks
TT = S // P     # 8 token tiles
MC = MLP // P   # 32 mlp chunks
VW = NH * (DH + 1)  # 1040
VH = VW // 2        # 520 (8 heads per half)
WS = 32.0           # host-side scale for fp8 weights (Wq, Wk)
W2S = 64.0          # host-side scale for fp8 W2

_NC = None
LAST_RESULTS = None


def _dram(nc, name, shape, dt, kind="ExternalInput"):
    return nc.dram_tensor(name, list(shape), dt, kind=kind).ap()


def build_nc():
    nc = bacc.Bacc("TRN2", target_bir_lowering=False, debug=False, num_devices=8)

    x_d = _dram(nc, "x", [S, H], F32)
    c_d = _dram(nc, "c", [1, H], F32)
    wc_d = _dram(nc, "wc", [H, 6 * H], BF16)
    bc_d = _dram(nc, "bc", [1, 6 * H], F32)
    wq_d = _dram(nc, "wq", [H, H], FP8)      # x WS
    wk_d = _dram(nc, "wk", [H, H], FP8)      # x WS
    wv_d = _dram(nc, "wv", [H, VW], BF16)    # per head [v64 | ones]
    wo_d = _dram(nc, "wo", [H, H], BF16)
    w1_d = _dram(nc, "w1", [H, MLP], BF16)
    w2_d = _dram(nc, "w2", [MLP, H], FP8)    # x W2S
    # packed bias tensors: bcols = [bq(8) | bk(8) | b1(32)] col-major [P,48];
    # brows = [bve(VW) | 16*bo(H) | 64*b2(H)] bf16 row
    bcols_d = _dram(nc, "bcols", [P, 48], F32)
    brows_d = _dram(nc, "brows", [1, VW + 2 * H], BF16)
    out_d = _dram(nc, "out", [S, H], F32, kind="ExternalOutput")

    wc3 = wc_d.rearrange("(kc p) n -> p kc n", p=P)
    wq3 = wq_d.rearrange("(kc p) n -> p kc n", p=P)
    wk3 = wk_d.rearrange("(kc p) n -> p kc n", p=P)
    wv3 = wv_d.rearrange("(kc p) n -> p kc n", p=P)
    wo3 = wo_d.rearrange("(kc p) n -> p kc n", p=P)
    w13 = w1_d.rearrange("(kc p) n -> p kc n", p=P)
    w23 = w2_d.rearrange("(kc p) n -> p kc n", p=P)

    es_wc = ExitStack()
    es_xm = ExitStack()
    es_attn = ExitStack()
    es_g = ExitStack()
    es_wqk = ExitStack()
    es_wv = ExitStack()
    es_wo = ExitStack()
    es_wo2 = ExitStack()

    with ExitStack() as es:
        tc = es.enter_context(tile.TileContext(nc))

        persist = es.enter_context(tc.tile_pool(name="persist", bufs=1))
        # PSUM (8 banks): sc = 2x[128,1024] f32 (4 banks); mm = 4x <=1 bank
        # (transposes, pv, den, c6, fc1).
        psum = es.enter_context(tc.tile_pool(name="psum", bufs=2, space="PSUM"))
        dramp = es.enter_context(tc.tile_pool(name="dram", bufs=1, space="DRAM"))
        pstat = es.enter_context(tc.tile_pool(name="stat", bufs=4))
        ptmp = es.enter_context(tc.tile_pool(name="tmp", bufs=2))
        pbm = es.enter_context(tc.tile_pool(name="bcmsa", bufs=1))
        pacts = es.enter_context(tc.tile_pool(name="acts", bufs=1))
        ph1 = es.enter_context(tc.tile_pool(name="h1p", bufs=1))
        pw2 = es.enter_context(tc.tile_pool(name="w2p", bufs=1))
        # attention-lifetime pools (LIFO: outlive wc/xm scopes)
        pattn = es_attn.enter_context(tc.tile_pool(name="attnp", bufs=1))
        ppt = es_attn.enter_context(tc.tile_pool(name="pthead", bufs=1))

        # ---------------- constants ----------------
        ident = persist.tile([P, P], BF16, name="ident")
        make_identity(nc, ident)
        eps_t = persist.tile([P, 1], F32, name="eps_t")
        nc.vector.memset(eps_t, EPS)
        ones_row = persist.tile([1, P], BF16, name="ones_row")
        nc.vector.memset(ones_row, 1.0)
        bcols = persist.tile([P, 48], F32, name="bcols")
        nc.sync.dma_start(out=bcols, in_=bcols_d)
        bq_t = bcols[:, 0:HC]
        bk_t = bcols[:, HC:2 * HC]
        b1_t = bcols[:, 2 * HC:2 * HC + MC]
        brows = persist.tile([1, VW + 2 * H], BF16, name="brows")
        nc.sync.dma_start(out=brows, in_=brows_d)
        bve_sb = brows[:, 0:VW]
        bor_sb = brows[:, VW:VW + H]
        b2r_sb = brows[:, VW + H:VW + 2 * H]

        # residual x first so its DMAs get queue priority
        x_res = persist.tile([P, TT * H], F32, name="x_res")
        x3 = x_d.rearrange("(i p) h -> p i h", p=P)
        for i2 in range(TT // 2):
            nc.sync.dma_start(
                out=x_res[:, i2 * 2 * H:(i2 + 1) * 2 * H].rearrange(
                    "p (i h) -> p i h", h=H),
                in_=x3[:, 2 * i2:2 * i2 + 2, :],
            )

        # ------------- adaLN: c6 = silu(c) @ Wc + bc (bf16) -------------
        ct = persist.tile([P, HC], F32, name="ct")
        nc.sync.dma_start(out=ct, in_=c_d.rearrange("o (j p) -> (o p) j", p=P))
        ct_b = persist.tile([P, HC], BF16, name="ct_b")
        nc.scalar.activation(out=ct_b, in_=ct, func=AF.Silu)
        c6_dram = dramp.tile([1, 6 * H], F32, name="c6_dram")

        pwc = es_wc.enter_context(tc.tile_pool(name="wcp", bufs=2))

        def c6_ntile(nt):
            ps = psum.tile([1, 512], F32, tag="mm", bufs=4, name="c6ps")
            for half in range(2):
                n0 = nt * 512 + half * 256
                wcs = pwc.tile([P, HC, 256], BF16, tag="wc", bufs=2,
                               name="wcs")
                nc.sync.dma_start(out=wcs, in_=wc3[:, :, n0:n0 + 256])
                for k in range(HC):
                    nc.tensor.matmul(
                        ps[:, half * 256:(half + 1) * 256],
                        lhsT=ct_b[:, k:k + 1], rhs=wcs[:, k, :],
                        start=(k == 0), stop=(k == HC - 1),
                    )
            bcrow = pwc.tile([1, 512], F32, tag="bcrow", bufs=1, name="bcrow")
            nc.sync.dma_start(out=bcrow, in_=bc_d[:, nt * 512:(nt + 1) * 512])
            stage = pwc.tile([1, 512], F32, tag="stage", bufs=1, name="stage")
            nc.vector.tensor_tensor(out=stage, in0=ps, in1=bcrow, op=ALU.add)
            nc.sync.dma_start(
                out=c6_dram[:, nt * 512:(nt + 1) * 512], in_=stage
            )

        for nt in range(4):  # sh_msa, sc_msa
            c6_ntile(nt)

        def cols_from_c6(pool, name, seg, plus1=False):
            t = pool.tile([P, HC], F32, name=name)
            nc.sync.dma_start(
                out=t,
                in_=c6_dram[:, seg * H:(seg + 1) * H].rearrange(
                    "o (j p) -> (o p) j", p=P
                ),
            )
            if plus1:
                nc.scalar.activation(out=t, in_=t, func=AF.Identity, bias=1.0)
            return t

        def ln_stats(src, mv):
            stats = pstat.tile([P, 2, 6], F32, tag="stats", name="stats")
            for sg in range(2):
                nc.vector.bn_stats(
                    out=stats[:, sg, :], in_=src[:, sg * 512:(sg + 1) * 512]
                )
            nc.vector.bn_aggr(out=mv, in_=stats)

        def ln_normalize(src, out_bf):
            mv = pstat.tile([P, 2], F32, tag="mv", name="mv")
            ln_stats(src, mv)
            sd = pstat.tile([P, 1], F32, tag="sd", name="sd")
            nc.scalar.activation(out=sd, in_=mv[:, 1:2], func=AF.Sqrt, bias=eps_t)
            rstd = pstat.tile([P, 1], F32, tag="rstd", name="rstd")
            nc.vector.reciprocal(rstd, sd)
            nmr = pstat.tile([P, 1], F32, tag="nmr", name="nmr")
            nc.vector.scalar_tensor_tensor(
                out=nmr, in0=mv[:, 0:1], scalar=-1.0, in1=rstd,
                op0=ALU.mult, op1=ALU.mult,
            )
            nc.scalar.activation(
                out=out_bf, in_=src, func=AF.Identity, bias=nmr, scale=rstd
            )

        def transpose_to(xm_b, dstT, i, scT, shT, act_mask=0xAA):
            # PE-transpose [128,128] chunks + modulate-copy; act_mask picks
            # ACT vs DVE per chunk to balance phase load.
            for hc in range(HC):
                tp = psum.tile([P, P], BF16, tag="mm", bufs=4, name="tp")
                nc.tensor.transpose(tp, xm_b[:, hc * P:(hc + 1) * P], ident)
                dst = dstT[:, hc, i * P:(i + 1) * P]
                if not (act_mask >> hc) & 1:
                    nc.vector.tensor_scalar(
                        out=dst, in0=tp,
                        scalar1=scT[:, hc:hc + 1], scalar2=shT[:, hc:hc + 1],
                        op0=ALU.mult, op1=ALU.add,
                    )
                else:
                    nc.scalar.activation(
                        out=dst, in_=tp, func=AF.Identity,
                        bias=shT[:, hc:hc + 1], scale=scT[:, hc:hc + 1],
                    )

        # outer-lifetime activations (bf16: feed bf16 GEMMs)
        yT = pacts.tile([P, HC, S], BF16, name="yT")
        xm2T = pacts.tile([P, HC, S], BF16, name="xm2T")

        shT_msa = pbm.tile([P, HC], F32, name="shT_msa")
        nc.sync.dma_start(
            out=shT_msa,
            in_=c6_dram[:, 0:H].rearrange("o (j p) -> (o p) j", p=P),
        )
        scT_msa = pbm.tile([P, HC], F32, name="scT_msa")
        nc.sync.dma_start(
            out=scT_msa,
            in_=c6_dram[:, H:2 * H].rearrange("o (j p) -> (o p) j", p=P),
        )
        nc.scalar.activation(out=scT_msa, in_=scT_msa, func=AF.Identity, bias=1.0)

        # ---- LN1 + modulate + transpose -> xmT (fp8) ----
        pxm = es_xm.enter_context(tc.tile_pool(name="xmp", bufs=1))
        xmT = pxm.tile([P, HC, S], FP8, name="xmT")
        for i in range(TT):
            xm_b = ptmp.tile([P, H], BF16, tag="xm_b", name="xm_b")
            ln_normalize(x_res[:, i * H:(i + 1) * H], xm_b)
            transpose_to(xm_b, xmT, i, scT_msa, shT_msa, act_mask=0x88)

        # attention-lifetime tensors: q/k in [32-row, 2-chunk] head layout
        # so scores can run DoubleRow (feature pairs split across chunk dim)
        q32 = pattn.tile([P, 4, 2, S], FP8, name="q32")
        k32 = pattn.tile([P, 4, 2, S], FP8, name="k32")
        v_sb = pattn.tile([P, TT, VW], FP8, name="v_sb")

        # ---- q^T / k^T (fp8 DoubleRow) + rearrange to 32x2 layout ----
        pwqk = es_wqk.enter_context(tc.tile_pool(name="wqkp", bufs=2))
        for ti, (w3, bias_t, dst32) in enumerate(
            ((wq3, bq_t, q32), (wk3, bk_t, k32))
        ):
            es_t = ExitStack()
            ptq = es_t.enter_context(tc.tile_pool(name=f"qk{ti}", bufs=1))
            for ocH in range(2):
                tT = ptq.tile([P, 4, S], FP8, tag="tT", bufs=1, name="tT")
                wh = pwqk.tile([P, HC, 512], FP8, tag="wqk", bufs=2,
                               name="wh")
                nc.sync.dma_start(
                    out=wh, in_=w3[:, :, ocH * 512:(ocH + 1) * 512]
                )
                for ocl in range(4):
                    oc = ocH * 4 + ocl
                    ps = psum.tile([P, 1024], F32, tag="sc", name="qkps")
                    for nh2 in range(2):
                        for kp in range(HC // 2):
                            nc.tensor.matmul(
                                ps[:, nh2 * 512:(nh2 + 1) * 512],
                                lhsT=wh[:, 2 * kp:2 * kp + 2,
                                        ocl * P:(ocl + 1) * P],
                                rhs=xmT[:, 2 * kp:2 * kp + 2,
                                        nh2 * 512:(nh2 + 1) * 512],
                                start=(kp == 0), stop=(kp == HC // 2 - 1),
                                perf_mode=DR,
                            )
                    nc.scalar.activation(
                        out=tT[:, ocl, :], in_=ps, func=AF.Identity,
                        bias=bias_t[:, oc:oc + 1], scale=1.0 / WS,
                    )
                # rearrange: head h features (64) -> 32 partitions x 2 chunks
                for h in range(8 * ocH, 8 * ocH + 8):
                    nc.sync.dma_start(
                        out=dst32[(h % 4) * 32:(h % 4) * 32 + 32,
                                  h // 4, :, :],
                        in_=tT[(h % 2) * DH:(h % 2) * DH + DH,
                               (h // 2) % 4, :],
                    )
            es_t.close()

        # ---- v (bf16 weights x fp8 xmT; token-major out, fp8) ----
        es_wqk.close()
        pwv = es_wv.enter_context(tc.tile_pool(name="wvp", bufs=1))
        for vh in range(2):
            wvh = pwv.tile([P, HC, VH], BF16, tag="wv", bufs=1, name="wvh")
            nc.sync.dma_start(out=wvh, in_=wv3[:, :, vh * VH:(vh + 1) * VH])
            for i in range(TT):
                ps = psum.tile([P, 1024], F32, tag="sc", name="vps")
                for (n0, n1) in ((0, 512), (512, VH)):
                    pss = ps[:, n0:n1]
                    for k in range(HC):
                        nc.tensor.matmul(
                            pss,
                            lhsT=xmT[:, k, i * P:(i + 1) * P],
                            rhs=wvh[:, k, n0:n1],
                            start=(k == 0), stop=False,
                        )
                    nc.tensor.matmul(
                        pss, lhsT=ones_row,
                        rhs=bve_sb[:, vh * VH + n0: vh * VH + n1],
                        start=False, stop=True,
                    )
                nc.scalar.activation(
                    out=v_sb[:, i, vh * VH:(vh + 1) * VH], in_=ps[:, 0:VH],
                    func=AF.Identity,
                )
        es_wv.close()
        es_xm.close()   # xmT dead

        def scores_units(qh, hc, pTs):
            """8 thunks: one [128,1024] scores psum + exp each."""
            q0 = qh * 512
            units = []
            for h2 in (0, 1):
                h = 2 * hc + h2
                p0 = (h % 4) * 32
                hg = h // 4
                for jp in range(4):
                    def u(h2=h2, p0=p0, hg=hg, jp=jp):
                        sp = psum.tile([P, 1024], F32, tag="sc", name="sps")
                        for jj in (0, 1):
                            j = 2 * jp + jj
                            nc.tensor.matmul(
                                sp[:, jj * 512:(jj + 1) * 512],
                                lhsT=k32[p0:p0 + 32, hg, :,
                                         j * P:(j + 1) * P],
                                rhs=q32[p0:p0 + 32, hg, :, q0:q0 + 512],
                                start=True, stop=True, perf_mode=DR,
                                tile_position=(p0, 0),
                            )
                        nc.scalar.activation(
                            out=pTs[h2][:, 2 * jp:2 * jp + 2, :], in_=sp,
                            func=AF.Exp, scale=1.0 / DH,
                        )
                    units.append(u)
            return units

        def pv_units(qh, hc, pTs, st):
            """8 thunks accumulating PV + den; st collects psum tiles."""
            units = []
            for h2 in (0, 1):
                h = 2 * hc + h2
                for jp in range(4):
                    def u(h2=h2, h=h, jp=jp):
                        if jp == 0 and h2 == 0:
                            st["pvy"] = psum.tile([P, 512], F32, tag="mm",
                                                  bufs=4, name="pvy")
                            st["pds"] = [
                                psum.tile([1, 512], F32, tag="mm", bufs=4,
                                          name="pd")
                                for _ in range(2)
                            ]
                        pvy, pds = st["pvy"], st["pds"]
                        if h2 == 0:
                            nc.tensor.matmul(
                                pvy[0:DH, :],
                                lhsT=v_sb[:, 2 * jp:2 * jp + 2,
                                          h * 65:h * 65 + DH],
                                rhs=pTs[h2][:, 2 * jp:2 * jp + 2, :],
                                start=(jp == 0), stop=(jp == 3), perf_mode=DR,
                            )
                        else:
                            for jj in (0, 1):
                                j = 2 * jp + jj
                                nc.tensor.matmul(
                                    pvy[DH:2 * DH, :],
                                    lhsT=v_sb[:, j, h * 65:h * 65 + DH],
                                    rhs=pTs[h2][:, j, :],
                                    start=(j == 0), stop=(j == 7),
                                )
                        nc.tensor.matmul(
                            pds[h2],
                            lhsT=v_sb[:, 2 * jp:2 * jp + 2,
                                      h * 65 + DH:h * 65 + DH + 1],
                            rhs=pTs[h2][:, 2 * jp:2 * jp + 2, :],
                            start=(jp == 0), stop=(jp == 3), perf_mode=DR,
                        )
                    units.append(u)
            return units

        def pv_finish(qh, hc, st):
            """reciprocal chain + normalized yT writes for a finished pair."""
            q0 = qh * 512
            pvy, pds = st["pvy"], st["pds"]
            rbs = []
            for h2 in (0, 1):
                dencp = ppt.tile([1, 512], F32, tag="dencp", bufs=1,
                                 name="dencp")
                nc.vector.tensor_copy(out=dencp, in_=pds[h2])
                nc.vector.reciprocal_approx_fast(out=dencp, in_=dencp)
                recipb = ppt.tile([P, 512], F32, tag="recipb", bufs=1,
                                  name="recipb")
                nc.gpsimd.partition_broadcast(recipb, dencp)
                rbs.append(recipb)
            for h2 in (0, 1):
                nc.vector.scalar_tensor_tensor(
                    out=yT[h2 * DH:(h2 + 1) * DH, hc, q0:q0 + 512],
                    in0=pvy[h2 * DH:(h2 + 1) * DH, :], scalar=0.25,
                    in1=rbs[h2][h2 * DH:(h2 + 1) * DH, :],
                    op0=ALU.mult, op1=ALU.mult,
                )

        def attn_phase(qh, extra_fn=None):
            """Software-pipelined attention: scores of pair hc interleave
            with PV of pair hc-1 so the PE never waits on the exp ring.
            extra_fn(hc) emits filler PE/ACT work after each iteration."""
            prev = None
            for hc in range(HC + 1):
                s_units, cur = [], None
                if hc < HC:
                    pTs = [
                        ppt.tile([P, TT, 512], FP8, tag="pT", bufs=4,
                                 name="pT")
                        for _ in range(2)
                    ]
                    s_units = scores_units(qh, hc, pTs)
                    cur = (hc, pTs, {})
                p_units = (pv_units(qh, prev[0], prev[1], prev[2])
                           if prev else [])
                for u in range(8):
                    if u < len(s_units):
                        s_units[u]()
                    if p_units:
                        p_units[u]()
                if prev:
                    pv_finish(qh, prev[0], prev[2])
                if extra_fn is not None and hc < HC:
                    extra_fn(hc)
                prev = cur

        # ---- attention query-half 0 (+ rest of c6 in the PE gaps) ----
        attn_phase(0, extra_fn=lambda hc: c6_ntile(4 + hc))
        es_wc.close()

        # gates (bf16 via f32 staging) and mlp modulate columns
        pg = es_g.enter_context(tc.tile_pool(name="gstage", bufs=1))
        g_msa = pbm.tile([P, H], BF16, name="g_msa")
        g_mlp = persist.tile([P, H], BF16, name="g_mlp")
        for gdst, seg in ((g_msa, 2), (g_mlp, 5)):
            gtmp = pg.tile([P, H], F32, tag="g", bufs=2, name="gtmp")
            nc.sync.dma_start(
                out=gtmp,
                in_=c6_dram[:, seg * H:(seg + 1) * H].to_broadcast([P, H]),
            )
            nc.scalar.activation(out=gdst, in_=gtmp, func=AF.Identity)
        shT_mlp = cols_from_c6(persist, "shT_mlp", 3)
        scT_mlp = cols_from_c6(persist, "scT_mlp", 4, plus1=True)
        es_g.close()

        # out-proj weights for half 0 (bf16; reloaded for half 1)
        pwo = es_attn.enter_context(tc.tile_pool(name="wop", bufs=2))
        woh = []
        for nh2 in range(2):
            w = pwo.tile([P, HC, 512], BF16, tag="wo", bufs=2, name="woh")
            nc.sync.dma_start(out=w, in_=wo3[:, :, nh2 * 512:(nh2 + 1) * 512])
            woh.append(w)

        def proj_tile(i, act_mask, woh):
            """out-proj (bf16) for token tile i + residual + LN2 + xm2T."""
            ps = psum.tile([P, 1024], F32, tag="sc", name="prps")
            for nh2 in range(2):
                pss = ps[:, nh2 * 512:(nh2 + 1) * 512]
                for k in range(HC):
                    nc.tensor.matmul(
                        pss,
                        lhsT=yT[:, k, i * P:(i + 1) * P],
                        rhs=woh[nh2][:, k, :],
                        start=(k == 0), stop=False,
                    )
                nc.tensor.matmul(
                    pss, lhsT=ones_row,
                    rhs=bor_sb[:, nh2 * 512:(nh2 + 1) * 512],
                    start=False, stop=True,
                )
            rt = ptmp.tile([P, H], F32, tag="rt", bufs=1, name="rt")
            nc.vector.scalar_tensor_tensor(
                out=rt, in0=ps, scalar=1.0 / 16.0, in1=g_msa,
                op0=ALU.mult, op1=ALU.mult,
            )
            xsl = x_res[:, i * H:(i + 1) * H]
            nc.vector.tensor_tensor(out=xsl, in0=xsl, in1=rt, op=ALU.add)
            xm_b = ptmp.tile([P, H], BF16, tag="xm_b", name="xm2_b")
            ln_normalize(xsl, xm_b)
            transpose_to(xm_b, xm2T, i, scT_mlp, shT_mlp, act_mask=act_mask)

        def fc1_chunk(h1h, w1s, half, mch):
            ml = mch % 4   # w1s covers mlp cols (mch//4)*512 .. +512
            ps = psum.tile([P, 512], F32, tag="mm", bufs=4, name="f1ps")
            for k in range(HC):
                nc.tensor.matmul(
                    ps,
                    lhsT=w1s[:, k, ml * P:(ml + 1) * P],
                    rhs=xm2T[:, k, half * 512:(half + 1) * 512],
                    start=(k == 0), stop=(k == HC - 1),
                )
            nc.scalar.activation(
                out=h1h[:, mch, :], in_=ps, func=AF.Gelu,
                bias=b1_t[:, mch:mch + 1],
            )

        def fc2_tile(h1h, half, il):
            """fc2 (fp8 DR) + gate + residual + store, token tile half*4+il."""
            it = half * 4 + il
            ps = psum.tile([P, 1024], F32, tag="sc", name="f2ps")
            for nh2 in range(2):
                pss = ps[:, nh2 * 512:(nh2 + 1) * 512]
                for mp in range(MC // 2):
                    nc.tensor.matmul(
                        pss,
                        lhsT=h1h[:, 2 * mp:2 * mp + 2, il * P:(il + 1) * P],
                        rhs=w2_sb[:, 2 * mp:2 * mp + 2,
                                  nh2 * 512:(nh2 + 1) * 512],
                        start=(mp == 0), stop=False, perf_mode=DR,
                    )
                nc.tensor.matmul(
                    pss, lhsT=ones_row,
                    rhs=b2r_sb[:, nh2 * 512:(nh2 + 1) * 512],
                    start=False, stop=True,
                )
            ot = pout.tile([P, H], F32, tag="ot", name="ot")
            nc.vector.scalar_tensor_tensor(
                out=ot, in0=ps, scalar=1.0 / W2S, in1=g_mlp,
                op0=ALU.mult, op1=ALU.mult,
            )
            nc.vector.tensor_tensor(
                out=ot, in0=ot, in1=x_res[:, it * H:(it + 1) * H], op=ALU.add
            )
            nc.sync.dma_start(out=out_d[it * P:(it + 1) * P, :], in_=ot)

        # fc2 weights (fp8, 4.2MB) - DMA during the interleaved phase
        w2_sb = pw2.tile([P, MC, H], FP8, name="w2_sb")
        for q4 in range(4):
            nc.sync.dma_start(
                out=w2_sb[:, 8 * q4:8 * q4 + 8, :],
                in_=w23[:, 8 * q4:8 * q4 + 8, :],
            )

        # ---- attention query-half 1 interleaved with proj/LN2 half 0
        # (hc 0-3) and fc1 half 0 (hc 4-7) ----
        pw1i = es_attn.enter_context(tc.tile_pool(name="w1pi", bufs=2))
        h1h0 = ph1.tile([P, MC, 512], FP8, tag="h1", bufs=1, name="h1h0")

        def qh1_filler(hc):
            if hc < 4:
                proj_tile(hc, act_mask=0x00, woh=woh)
            else:
                q8 = 2 * (hc - 4)
                for qq in (q8, q8 + 1):
                    w1s = pw1i.tile([P, HC, 512], BF16, tag="w1s", bufs=2,
                                    name="w1s")
                    nc.sync.dma_start(
                        out=w1s, in_=w13[:, :, qq * 512:(qq + 1) * 512]
                    )
                    for m4 in range(4):
                        fc1_chunk(h1h0, w1s, 0, 4 * qq + m4)

        attn_phase(1, extra_fn=qh1_filler)
        es_attn.close()   # q32/k32/v_sb/pT/w1s dead
        pout = es.enter_context(tc.tile_pool(name="outp", bufs=2))

        # ---- proj/LN2 half 1 interleaved with fc2 half 0 ----
        pwo2 = es_wo2.enter_context(tc.tile_pool(name="wop2", bufs=2))
        woh2 = []
        for nh2 in range(2):
            w = pwo2.tile([P, HC, 512], BF16, tag="wo", bufs=2, name="woh2")
            nc.sync.dma_start(out=w, in_=wo3[:, :, nh2 * 512:(nh2 + 1) * 512])
            woh2.append(w)
        for i in range(4):
            proj_tile(4 + i, act_mask=0xAA, woh=woh2)
            fc2_tile(h1h0, 0, i)
        es_wo2.close()

        # ---- fc1 half 1, then fc2 half 1 ----
        pw1b = es.enter_context(tc.tile_pool(name="w1pb", bufs=2))
        h1h1 = ph1.tile([P, MC, 512], FP8, tag="h1", bufs=1, name="h1h1")
        for q8 in range(8):
            w1s = pw1b.tile([P, HC, 512], BF16, tag="w1s", bufs=2,
                            name="w1s")
            nc.sync.dma_start(
                out=w1s, in_=w13[:, :, q8 * 512:(q8 + 1) * 512]
            )
            for ml in range(4):
                fc1_chunk(h1h1, w1s, 1, q8 * 4 + ml)
        for il in range(4):
            fc2_tile(h1h1, 1, il)

    nc.compile()
    return nc


def get_nc():
    global _NC
    if _NC is None:
        _NC = build_nc()
    return _NC


def _fp8(a):
    return np.clip(np.asarray(a, np.float32), -240.0, 240.0).astype(
        ml_dtypes.float8_e4m3
    )


def make_in_maps(inputs):
    bf = ml_dtypes.bfloat16
    x = np.ascontiguousarray(inputs["x"], dtype=np.float32)
    c = np.ascontiguousarray(inputs["c"], dtype=np.float32)
    Wv = np.asarray(inputs["Wv"], dtype=np.float32)
    bv = np.asarray(inputs["bv"], dtype=np.float32)
    wv_ext = np.zeros((H, VW), dtype=np.float32)
    bv_ext = np.zeros((1, VW), dtype=np.float32)
    for h in range(NH):
        wv_ext[:, h * 65:h * 65 + DH] = Wv[:, h * DH:(h + 1) * DH]
        bv_ext[0, h * 65:h * 65 + DH] = bv[h * DH:(h + 1) * DH]
        bv_ext[0, h * 65 + DH] = 1.0 / 64.0   # ones col -> den/64 in psum

    bcols = np.zeros((P, 48), np.float32)
    bcols[:, 0:HC] = np.asarray(inputs["bq"], np.float32).reshape(HC, P).T
    bcols[:, HC:2 * HC] = np.asarray(inputs["bk"], np.float32).reshape(HC, P).T
    bcols[:, 2 * HC:2 * HC + MC] = (
        np.asarray(inputs["b1"], np.float32).reshape(MC, P).T
    )
    brows = np.zeros((1, VW + 2 * H), np.float32)
    brows[0, 0:VW] = bv_ext[0]
    brows[0, VW:VW + H] = np.asarray(inputs["bo"], np.float32) * 16.0
    brows[0, VW + H:] = np.asarray(inputs["b2"], np.float32) * W2S
    shared = {
        "wc": np.asarray(inputs["Wc"], np.float32).astype(bf),
        "bc": np.asarray(inputs["bc"], np.float32).reshape(1, 6 * H),
        "wq": _fp8(np.asarray(inputs["Wq"], np.float32) * WS),
        "wk": _fp8(np.asarray(inputs["Wk"], np.float32) * WS),
        "wv": wv_ext.astype(bf),
        "wo": np.asarray(inputs["Wo"], np.float32).astype(bf),
        "w1": np.asarray(inputs["W1"], np.float32).astype(bf),
        "w2": _fp8(np.asarray(inputs["W2"], np.float32) * W2S),
        "bcols": bcols,
        "brows": brows.astype(bf),
    }
    in_maps = []
    for b in range(B):
        m = dict(shared)
        m["x"] = x[b]
        m["c"] = c[b:b + 1]
        in_maps.append(m)
    return in_maps


def kernel(**inputs) -> np.ndarray:
    global LAST_RESULTS
    nc = get_nc()
    in_maps = make_in_maps(inputs)
    res = bass_utils.run_bass_kernel_spmd(nc, in_maps, core_ids=list(range(B)))
    LAST_RESULTS = res
    out = np.stack([res.results[b]["out"] for b in range(B)], axis=0)
    return out.astype(np.float32)


if __name__ == "__main__":
    build_nc()
    print("built and compiled OK")
